# revision 9
# baseline (speedup 1.0000x reference)
"""Trainium2 Bass kernel for 2-layer multi-head GAT (nn_GAT_82867099009055).

Sharding: edges are sharded by DESTINATION range across the 8 cores, so each
dst node's whole in-neighborhood (softmax denominator + weighted sum) is
core-local. Per layer a per-node message table (bf16) and attention-scalar
table (fp32) are built by dense matmuls; the edge phase gathers table rows by
src (dma_gather; int16 indices force a low/high split at 32768) and scalar
rows by dst from a core-local table. Segment-sum runs on the PE: per 128-edge
block a one-hot matrix M[j,v] = (dst_local[j]==v) built by one DVE is_equal
feeds psum[v,:] += M.T @ (e*msg), accumulating a 128-dst tile in PSUM; a
second matmul with rhs=e accumulates softmax denominators. Softmax
max-subtract is dropped (scores bounded, ratio invariant; the reference's
+1e-10 denominator epsilon differs by ~1e-6 relative). BatchNorm batch stats
come from ones-vector matmuls all-reduced across cores (~4KB). Layer-2 tables
are computed on owned rows and all-gathered. Output is the owned dst slice,
concatenated on the host.
"""

import sys

for _p in ("/opt/trn_rl_repo",):
    if _p not in sys.path:
        sys.path.insert(0, _p)

from dataclasses import dataclass, field

import numpy as np

import concourse.bass as bass
import concourse.mybir as mybir
import concourse.tile as tile
from concourse import bacc
from concourse.masks import make_identity

FP32 = mybir.dt.float32
BF16 = mybir.dt.bfloat16
I16 = mybir.dt.int16
AX = mybir.AluOpType
ACT = mybir.ActivationFunctionType


@dataclass
class Cfg:
    N: int = 50000
    E: int = 1600000
    DIN: int = 128
    DH: int = 64
    H: int = 8
    DOUT: int = 128
    CORES: int = 8
    ALPHA: float = 0.2
    BN_EPS: float = 1e-5
    SPLIT: int = 32768          # int16 index range per gather call
    CHUNK: int = 4              # blocks per DVE build chunk
    GBLK: int = 8               # blocks per gather call

    @property
    def NLOC(self):
        return self.N // self.CORES

    @property
    def NTILES(self):
        return (self.NLOC + 127) // 128

    @property
    def NLOCP(self):
        return self.NTILES * 128

    @property
    def D1(self):
        return self.H * self.DH


@dataclass
class Sched:
    nb: list = field(default_factory=list)       # [t][h] -> #128-edge blocks
    run_off: list = field(default_factory=list)  # [t][h] -> block offset
    tile_off: list = field(default_factory=list)
    total_blocks: int = 0


def _wrap16(idx, P=128):
    n = idx.shape[0]
    assert n % 16 == 0
    w = idx.reshape(n // 16, 16).T.astype(np.int16)
    return np.ascontiguousarray(np.tile(w, (P // 16, 1)))


def host_prep(cfg, feat, edges, W_heads, a_heads, gamma_h, beta_h,
              W_out, a_out, gamma_o, beta_o, W_res, b_res):
    c = cfg
    src = edges[0].astype(np.int64)
    dst = edges[1].astype(np.int64)
    core_of = dst // c.NLOC

    per_core = []
    cnts = np.zeros((c.CORES, c.NTILES, 2), np.int64)
    for ci in range(c.CORES):
        m = core_of == ci
        s, d = src[m], dst[m]
        dl = d - ci * c.NLOC
        t = dl // 128
        h = (s >= c.SPLIT).astype(np.int64)
        order = np.lexsort((h, t))
        s, dl, t, h = s[order], dl[order], t[order], h[order]
        per_core.append((s, dl, t, h))
        for ti in range(c.NTILES):
            tm = t == ti
            cnts[ci, ti, 0] = int(np.sum(tm & (h == 0)))
            cnts[ci, ti, 1] = int(np.sum(tm & (h == 1)))

    sch = Sched()
    sch.nb = [[int(max(1, -(-int(cnts[:, ti, hh].max()) // 128)))
               for hh in range(2)] for ti in range(c.NTILES)]
    off = 0
    for ti in range(c.NTILES):
        sch.tile_off.append(off)
        sch.run_off.append([off, off + sch.nb[ti][0]])
        off += sch.nb[ti][0] + sch.nb[ti][1]
    sch.total_blocks = off
    TB = sch.total_blocks
    TS = TB * 128

    iota = np.tile(np.arange(128, dtype=np.float32), (128, 1))
    W1cat = np.concatenate([W_heads[hh] for hh in range(c.H)], axis=1)
    a1s = np.stack([W_heads[hh].astype(np.float64) @ a_heads[hh][:c.DH].astype(np.float64)
                    for hh in range(c.H)], 1).astype(np.float32)
    a1d = np.stack([W_heads[hh].astype(np.float64) @ a_heads[hh][c.DH:].astype(np.float64)
                    for hh in range(c.H)], 1).astype(np.float32)
    W_all1 = np.zeros((c.DIN, c.D1 + 16), np.float32)
    W_all1[:, :c.D1] = W1cat
    W_all1[:, c.D1:c.D1 + c.H] = a1s
    W_all1[:, c.D1 + 8:c.D1 + 8 + c.H] = a1d

    a2s = (W_out.astype(np.float64) @ a_out[:c.DOUT].astype(np.float64)).astype(np.float32)
    a2d = (W_out.astype(np.float64) @ a_out[c.DOUT:].astype(np.float64)).astype(np.float32)
    W2T = np.concatenate([W_out.astype(np.float32), a2s[:, None], a2d[:, None]], axis=1)

    g1b1 = np.concatenate([gamma_h.reshape(-1), beta_h.reshape(-1)]).astype(np.float32)[None, :]
    g2b2 = np.concatenate([gamma_o, beta_o]).astype(np.float32)[None, :]
    featT = np.ascontiguousarray(feat.T.astype(np.float32))

    in_maps = []
    for ci in range(c.CORES):
        s, dl, t, h = per_core[ci]
        src_idx = np.zeros(TS, np.int64)
        dloc = np.full(TS, -1.0, np.float32)
        dloc_core = np.zeros(TS, np.int64)
        for ti in range(c.NTILES):
            for hh in range(2):
                m = (t == ti) & (h == hh)
                n = int(m.sum())
                base = sch.run_off[ti][hh] * 128
                src_idx[base:base + n] = s[m] - (c.SPLIT if hh else 0)
                dloc[base:base + n] = (dl[m] - ti * 128).astype(np.float32)
                dloc_core[base:base + n] = dl[m]
        fown = np.zeros((c.DIN, c.NLOCP), np.float32)
        fown[:, :c.NLOC] = featT[:, ci * c.NLOC:(ci + 1) * c.NLOC]
        in_maps.append({
            "featT": featT,
            "featT_own": fown,
            "W_all1": W_all1,
            "W2T": W2T,
            "Wres": W_res.astype(np.float32),
            "bres_rep": np.ascontiguousarray(np.tile(b_res.astype(np.float32)[None, :], (128, 1))),
            "g1b1": g1b1,
            "g2b2": g2b2,
            "iota_rep": iota,
            "gidx_src": _wrap16(src_idx.astype(np.int16)),
            "gidx_dst": _wrap16(dloc_core.astype(np.int16)),
            "dstloc": np.ascontiguousarray(dloc.reshape(TB, 128).T),
        })
    return in_maps, sch


def build_module(cfg, sch):
    c = cfg
    TB = sch.total_blocks
    D1, DO = c.D1, c.DOUT
    NCH1 = D1 // 128
    nc = bacc.Bacc("TRN2", target_bir_lowering=False, debug=False,
                   enable_asserts=False, num_devices=c.CORES)

    featT = nc.dram_tensor("featT", [c.DIN, c.N], FP32, kind="ExternalInput")
    featT_own = nc.dram_tensor("featT_own", [c.DIN, c.NLOCP], FP32, kind="ExternalInput")
    W_all1 = nc.dram_tensor("W_all1", [c.DIN, D1 + 16], FP32, kind="ExternalInput")
    W2T = nc.dram_tensor("W2T", [D1, DO + 2], FP32, kind="ExternalInput")
    Wres = nc.dram_tensor("Wres", [c.DIN, DO], FP32, kind="ExternalInput")
    bres_rep = nc.dram_tensor("bres_rep", [128, DO], FP32, kind="ExternalInput")
    g1b1 = nc.dram_tensor("g1b1", [1, 2 * D1], FP32, kind="ExternalInput")
    g2b2 = nc.dram_tensor("g2b2", [1, 2 * DO], FP32, kind="ExternalInput")
    iota_rep = nc.dram_tensor("iota_rep", [128, 128], FP32, kind="ExternalInput")
    gidx_src = nc.dram_tensor("gidx_src", [128, TB * 8], I16, kind="ExternalInput")
    gidx_dst = nc.dram_tensor("gidx_dst", [128, TB * 8], I16, kind="ExternalInput")
    dstloc = nc.dram_tensor("dstloc", [128, TB], FP32, kind="ExternalInput")
    out = nc.dram_tensor("out", [c.NLOCP, DO], FP32, kind="ExternalOutput")

    tab1 = nc.dram_tensor("tab1", [c.N, D1], BF16)
    asad1 = nc.dram_tensor("asad1", [c.N, 64], FP32)
    adloc1 = nc.dram_tensor("adloc1", [c.NLOCP, 64], FP32)
    x1pre = nc.dram_tensor("x1pre", [c.NLOCP, D1], FP32)
    tab2mine = nc.dram_tensor("tab2mine", [c.NLOC, DO], BF16)
    asad2mine = nc.dram_tensor("asad2mine", [c.NLOC, 64], FP32)
    tab2 = nc.dram_tensor("tab2", [c.N, DO], BF16)
    asad2 = nc.dram_tensor("asad2", [c.N, 64], FP32)
    adloc2 = nc.dram_tensor("adloc2", [c.NLOCP, 64], FP32)

    NGT = (c.N + 127) // 128
    GROUPS = [(0, c.SPLIT), (c.SPLIT, c.N)]
    rg = [list(range(c.CORES))]

    def r3(ap, k):
        return ap.rearrange("p (b k) -> p b k", k=k)

    with tile.TileContext(nc) as tc:
        with (
            tc.tile_pool(name="const", bufs=1) as cpool,
            tc.tile_pool(name="setup", bufs=3) as spool,
            tc.tile_pool(name="stage", bufs=2) as gpool,
            tc.tile_pool(name="work", bufs=3) as wpool,
            tc.tile_pool(name="keep", bufs=1) as kpool,
            tc.tile_pool(name="bn", bufs=1) as bnpool,
            tc.tile_pool(name="pe", bufs=2, space="PSUM") as pe_pool,
            tc.tile_pool(name="pstat", bufs=1, space="PSUM") as pstat_pool,
            tc.tile_pool(name="pmisc", bufs=2, space="PSUM") as pmisc_pool,
            tc.tile_pool(name="dram", bufs=1, space="DRAM") as dpool,
        ):
            # ---------------- constants ----------------
            w1_sb = cpool.tile([c.DIN, D1 + 16], FP32)
            nc.sync.dma_start(w1_sb[:], W_all1[:, :])
            iota_sb = cpool.tile([128, 128], FP32)
            nc.sync.dma_start(iota_sb[:], iota_rep[:, :])
            w2_sb = cpool.tile([128, NCH1 * (DO + 2)], FP32)
            for ch in range(NCH1):
                nc.sync.dma_start(w2_sb[:, ch * (DO + 2):(ch + 1) * (DO + 2)],
                                  W2T[ch * 128:(ch + 1) * 128, :])
            wres_sb = cpool.tile([c.DIN, DO], FP32)
            nc.sync.dma_start(wres_sb[:], Wres[:, :])
            bres_sb = cpool.tile([128, DO], FP32)
            nc.sync.dma_start(bres_sb[:], bres_rep[:, :])
            g1_sb = cpool.tile([1, 2 * D1], FP32)
            nc.sync.dma_start(g1_sb[:], g1b1[:, :])
            g2_sb = cpool.tile([1, 2 * DO], FP32)
            nc.sync.dma_start(g2_sb[:], g2b2[:, :])
            ident = cpool.tile([128, 128], FP32)
            make_identity(nc, ident[:])
            ones_col = cpool.tile([128, 1], FP32)
            nc.vector.memset(ones_col[:], 1.0)
            ones_row = cpool.tile([1, 128], FP32)
            nc.vector.memset(ones_row[:], 1.0)

            # ---------------- layer-1 tables (all nodes, replicated) ----------------
            for g in range(NGT):
                n0 = g * 128
                cnt = min(c.N, n0 + 128) - n0
                lhsT = spool.tile([c.DIN, 128], FP32, tag="ft")
                nc.sync.dma_start(lhsT[:, :cnt], featT[:, n0:n0 + cnt])
                ps1 = pe_pool.tile([128, D1], FP32, tag="pA")
                ps2 = pe_pool.tile([128, 16], FP32, tag="pB")
                nc.tensor.matmul(ps1[:cnt, :], lhsT[:, :cnt], w1_sb[:, 0:D1],
                                 start=True, stop=True)
                nc.tensor.matmul(ps2[:cnt, :], lhsT[:, :cnt], w1_sb[:, D1:D1 + 16],
                                 start=True, stop=True)
                mbf = spool.tile([128, D1], BF16, tag="mbf")
                nc.vector.tensor_copy(mbf[:cnt, :], ps1[:cnt, :])
                asv = spool.tile([128, 64], FP32, tag="asv")
                nc.vector.memset(asv[:], 0.0)
                nc.vector.tensor_copy(asv[:cnt, 0:16], ps2[:cnt, :])
                nc.sync.dma_start(tab1[n0:n0 + cnt, :], mbf[:cnt, :])
                nc.sync.dma_start(asad1[n0:n0 + cnt, :], asv[:cnt, :])

            for t in range(c.NTILES):
                lhsT = spool.tile([c.DIN, 128], FP32, tag="ft")
                nc.sync.dma_start(lhsT[:], featT_own[:, t * 128:(t + 1) * 128])
                ps2 = pe_pool.tile([128, 16], FP32, tag="pB")
                nc.tensor.matmul(ps2[:], lhsT[:], w1_sb[:, D1:D1 + 16],
                                 start=True, stop=True)
                asv = spool.tile([128, 64], FP32, tag="asv")
                nc.vector.memset(asv[:], 0.0)
                nc.vector.tensor_copy(asv[:, 0:16], ps2[:])
                nc.sync.dma_start(adloc1[t * 128:(t + 1) * 128, :], asv[:])

            # ---------------- edge phase (shared for both layers) ----------------
            def edge_phase(tab, asad, adloc, dh_all, nheads, stats_x, stats_sq,
                           x_store, x_finish):
                dhh = dh_all // nheads
                GB = c.GBLK
                for t in range(c.NTILES):
                    nbl, nbh = sch.nb[t]
                    nbt = nbl + nbh
                    tb0 = sch.tile_off[t]
                    dl_t = gpool.tile([128, nbt], FP32, tag="dl")
                    nc.sync.dma_start(dl_t[:], dstloc[:, tb0:tb0 + nbt])
                    e_t = wpool.tile([128, nbt * nheads], FP32, tag="e")
                    psA = pe_pool.tile([128, dh_all], FP32, tag="pA")
                    psB = pe_pool.tile([128, nheads], FP32, tag="pB")
                    blk = 0
                    for hh, nb in enumerate((nbl, nbh)):
                        ro = 0 if hh == 0 else nbl
                        b0 = sch.run_off[t][hh]
                        lo, hi = GROUPS[hh]
                        for c0 in range(0, nb, GB):
                            cg = min(GB, nb - c0)
                            sl0 = b0 + c0          # block idx in gidx_src/dstloc
                            tl0 = ro + c0          # block idx within tile
                            gi = gpool.tile([128, cg * 8], I16, tag="gis")
                            nc.sync.dma_start(
                                gi[:], gidx_src[:, sl0 * 8:(sl0 + cg) * 8])
                            mst = gpool.tile([128, cg * dh_all], BF16, tag="ms")
                            nc.gpsimd.dma_gather(r3(mst[:], dh_all), tab[lo:hi, :],
                                                 gi[:], cg * 128, cg * 128, dh_all)
                            ast = gpool.tile([128, cg * 64], FP32, tag="as")
                            nc.gpsimd.dma_gather(r3(ast[:], 64), asad[lo:hi, :],
                                                 gi[:], cg * 128, cg * 128, 64)
                            gid = gpool.tile([128, cg * 8], I16, tag="gid")
                            nc.sync.dma_start(
                                gid[:], gidx_dst[:, (tb0 + tl0) * 8:(tb0 + tl0 + cg) * 8])
                            adt = gpool.tile([128, cg * 64], FP32, tag="ad")
                            nc.gpsimd.dma_gather(r3(adt[:], 64), adloc[:, :],
                                                 gid[:], cg * 128, cg * 128, 64)
                            # scores -> e for this gather chunk
                            ev = r3(e_t[:], nheads)[:, tl0:tl0 + cg, :]
                            nc.vector.tensor_tensor(
                                ev, r3(ast[:], 64)[:, :, 0:nheads],
                                r3(adt[:], 64)[:, :, 8:8 + nheads], op=AX.add)
                            nc.vector.scalar_tensor_tensor(ev, ev, c.ALPHA, ev,
                                                           op0=AX.mult, op1=AX.max)
                            nc.scalar.activation(ev, ev, ACT.Exp)
                            for c1 in range(0, cg, c.CHUNK):
                                cn = min(c.CHUNK, cg - c1)
                                mt = wpool.tile([128, c.CHUNK * 128], FP32, tag="M")
                                mv = r3(mt[:], 128)[:, 0:cn, :]
                                dsl = (dl_t[:, tl0 + c1:tl0 + c1 + cn]
                                       .rearrange("p (b o) -> p b o", o=1)
                                       .to_broadcast([128, cn, 128]))
                                iot = (iota_sb[:].rearrange("p (o v) -> p o v", o=1)
                                       .to_broadcast([128, cn, 128]))
                                nc.vector.tensor_tensor(mv, dsl, iot, op=AX.is_equal)
                                rhs = wpool.tile([128, c.CHUNK * dh_all], FP32, tag="rhs")
                                rv = (r3(rhs[:], dh_all)[:, 0:cn, :]
                                      .rearrange("p b (h k) -> p b h k", k=dhh))
                                msrc = (r3(mst[:], dh_all)[:, c1:c1 + cn, :]
                                        .rearrange("p b (h k) -> p b h k", k=dhh))
                                ein = (r3(e_t[:], nheads)[:, tl0 + c1:tl0 + c1 + cn, :]
                                       .rearrange("p b (h o) -> p b h o", o=1)
                                       .to_broadcast([128, cn, nheads, dhh]))
                                nc.vector.tensor_tensor(rv, msrc, ein, op=AX.mult)
                                for j in range(cn):
                                    first, last = blk == 0, blk == nbt - 1
                                    lt = r3(mt[:], 128)[:, j, :]
                                    nc.tensor.matmul(
                                        psA[:], lt, r3(rhs[:], dh_all)[:, j, :],
                                        start=first, stop=last, skip_group_check=True)
                                    nc.tensor.matmul(
                                        psB[:], lt,
                                        r3(e_t[:], nheads)[:, tl0 + c1 + j, :],
                                        start=first, stop=last, skip_group_check=True)
                                    blk += 1

                    den = wpool.tile([128, nheads], FP32, tag="den")
                    nc.vector.tensor_scalar_add(den[:], psB[:], 1e-10)
                    rec = wpool.tile([128, nheads], FP32, tag="rec")
                    nc.vector.reciprocal(rec[:], den[:])
                    xp = x_store(t)
                    nc.vector.tensor_tensor(
                        xp.rearrange("p (h k) -> p h k", k=dhh),
                        r3(psA[:], dhh),
                        rec[:].rearrange("p (h o) -> p h o", o=1)
                        .to_broadcast([128, nheads, dhh]),
                        op=AX.mult)
                    sq = wpool.tile([128, dh_all], FP32, tag="sq")
                    nc.vector.tensor_tensor(sq[:], xp, xp, op=AX.mult)
                    t0, t1 = t == 0, t == c.NTILES - 1
                    for h0 in range(0, dh_all, 512):
                        h1 = min(dh_all, h0 + 512)
                        nc.tensor.matmul(stats_x[0:1, h0:h1], ones_col[:],
                                         xp[:, h0:h1], start=t0, stop=t1,
                                         skip_group_check=True)
                        nc.tensor.matmul(stats_sq[0:1, h0:h1],
                                         ones_col[:], sq[:, h0:h1], start=t0,
                                         stop=t1, skip_group_check=True)
                    x_finish(t, xp)

            # ---------------- BN: stats -> replicated scale/shift ----------------
            def bn_scale_shift(stats_x, stats_sq, g_sb, dch):
                sb = bnpool.tile([1, 2 * dch], FP32, tag="bns")
                nc.vector.tensor_copy(sb[:, 0:dch], stats_x[0:1, 0:dch])
                nc.vector.tensor_copy(sb[:, dch:], stats_sq[0:1, 0:dch])
                bi = dpool.tile([1, 2 * dch], FP32, tag="bnb")
                bo = dpool.tile([1, 2 * dch], FP32, tag="bnb2")
                nc.sync.dma_start(bi[:], sb[:])  # [x-sums | sq-sums]
                nc.gpsimd.collective_compute("AllReduce", AX.add, replica_groups=rg,
                                             ins=[bi.opt()], outs=[bo.opt()])
                gs = bnpool.tile([1, 2 * dch], FP32, tag="bng")
                nc.sync.dma_start(gs[:], bo[:])
                mean = bnpool.tile([1, dch], FP32, tag="bnm")
                nc.vector.tensor_scalar_mul(mean[:], gs[:, 0:dch], 1.0 / c.N)
                m2 = bnpool.tile([1, dch], FP32, tag="bnm2")
                nc.vector.tensor_scalar_mul(m2[:], gs[:, dch:], 1.0 / c.N)
                var = bnpool.tile([1, dch], FP32, tag="bnv")
                nc.vector.tensor_tensor(var[:], mean[:], mean[:], op=AX.mult)
                nc.vector.tensor_tensor(var[:], m2[:], var[:], op=AX.subtract)
                nc.vector.tensor_scalar_add(var[:], var[:], c.BN_EPS)
                sd = bnpool.tile([1, dch], FP32, tag="bnsd")
                nc.scalar.activation(sd[:], var[:], ACT.Sqrt)
                rs = bnpool.tile([1, dch], FP32, tag="bnrs")
                nc.vector.reciprocal(rs[:], sd[:])
                sc = bnpool.tile([1, 2 * dch], FP32, tag="bnsc")
                nc.vector.tensor_tensor(sc[:, 0:dch], g_sb[:, 0:dch], rs[:], op=AX.mult)
                nc.vector.tensor_tensor(sc[:, dch:], mean[:], sc[:, 0:dch], op=AX.mult)
                nc.vector.tensor_tensor(sc[:, dch:], g_sb[:, dch:], sc[:, dch:],
                                        op=AX.subtract)
                rep = kpool.tile([128, 2 * dch], FP32, tag=f"bnrep{dch}")
                for h0 in range(0, 2 * dch, 512):
                    h1 = min(2 * dch, h0 + 512)
                    psr = pmisc_pool.tile([128, 512], FP32, tag="mx")
                    nc.tensor.matmul(psr[:, 0:h1 - h0], ones_row[:], sc[:, h0:h1],
                                     start=True, stop=True, skip_group_check=True)
                    nc.vector.tensor_copy(rep[:, h0:h1], psr[:, 0:h1 - h0])
                return rep

            # ---------------- layer 1 ----------------
            stats1x = pstat_pool.tile([1, D1], FP32, tag="stx")
            stats1q = pstat_pool.tile([1, D1], FP32, tag="stq")

            def store1(t):
                xs = spool.tile([128, D1], FP32, tag="x1t")
                return xs[:]

            def finish1(t, xp):
                nc.sync.dma_start(x1pre[t * 128:(t + 1) * 128, :], xp)

            edge_phase(tab1, asad1, adloc1, D1, c.H, stats1x[:], stats1q[:], store1, finish1)
            rep1 = bn_scale_shift(stats1x[:], stats1q[:], g1_sb, D1)

            # ---------------- layer 2 prep ----------------
            for t in range(c.NTILES):
                n0 = t * 128
                cnt = min(128, c.NLOC - n0)
                xp = spool.tile([128, D1], FP32, tag="x1t")
                nc.sync.dma_start(xp[:], x1pre[n0:n0 + 128, :])
                xb = spool.tile([128, D1], FP32, tag="x1b")
                nc.vector.tensor_tensor(xb[:], xp[:], rep1[:, 0:D1], op=AX.mult)
                nc.vector.tensor_tensor(xb[:], xb[:], rep1[:, D1:], op=AX.add)
                xtT = spool.tile([128, D1], FP32, tag="xtT")
                for ch in range(NCH1):
                    pst = pmisc_pool.tile([128, 512], FP32, tag="mx")
                    nc.tensor.transpose(pst[:, 0:128], xb[:, ch * 128:(ch + 1) * 128], ident[:])
                    nc.vector.tensor_copy(xtT[:, ch * 128:(ch + 1) * 128], pst[:, 0:128])
                ps_l2 = pmisc_pool.tile([128, 512], FP32, tag="mx")
                for ch in range(NCH1):
                    nc.tensor.matmul(ps_l2[:, 0:DO + 2], xtT[:, ch * 128:(ch + 1) * 128],
                                     w2_sb[:, ch * (DO + 2):(ch + 1) * (DO + 2)],
                                     start=(ch == 0), stop=(ch == NCH1 - 1),
                                     skip_group_check=True)
                m2bf = spool.tile([128, DO], BF16, tag="m2bf")
                nc.vector.tensor_copy(m2bf[:], ps_l2[:, 0:DO])
                as2 = spool.tile([128, 64], FP32, tag="asv")
                nc.vector.memset(as2[:], 0.0)
                nc.vector.tensor_copy(as2[:, 0:1], ps_l2[:, DO:DO + 1])
                nc.vector.tensor_copy(as2[:, 8:9], ps_l2[:, DO + 1:DO + 2])
                nc.sync.dma_start(tab2mine[n0:n0 + cnt, :], m2bf[:cnt, :])
                nc.sync.dma_start(asad2mine[n0:n0 + cnt, :], as2[:cnt, :])
                nc.sync.dma_start(adloc2[n0:n0 + 128, :], as2[:])

            nc.gpsimd.collective_compute("AllGather", AX.bypass, replica_groups=rg,
                                         ins=[tab2mine[:, :]], outs=[tab2[:, :]])
            nc.gpsimd.collective_compute("AllGather", AX.bypass, replica_groups=rg,
                                         ins=[asad2mine[:, :]], outs=[asad2[:, :]])

            # ---------------- layer 2 ----------------
            stats2x = pstat_pool.tile([1, DO], FP32, tag="stx")
            stats2q = pstat_pool.tile([1, DO], FP32, tag="stq")
            x2keep = kpool.tile([128, c.NTILES * DO], FP32, tag="x2")

            def store2(t):
                return x2keep[:, t * DO:(t + 1) * DO]

            def finish2(t, xp):
                pass

            edge_phase(tab2, asad2, adloc2, DO, 1, stats2x[:], stats2q[:], store2, finish2)
            rep2 = bn_scale_shift(stats2x[:], stats2q[:], g2_sb, DO)

            # ---------------- finalize ----------------
            for t in range(c.NTILES):
                n0 = t * 128
                lhsT = spool.tile([c.DIN, 128], FP32, tag="ft")
                nc.sync.dma_start(lhsT[:], featT_own[:, n0:n0 + 128])
                psR = pmisc_pool.tile([128, 512], FP32, tag="mx")
                nc.tensor.matmul(psR[:, 0:DO], lhsT[:], wres_sb[:], start=True,
                                 stop=True, skip_group_check=True)
                o = spool.tile([128, DO], FP32, tag="o")
                nc.vector.tensor_tensor(o[:], store2(t), rep2[:, 0:DO], op=AX.mult)
                nc.vector.tensor_tensor(o[:], o[:], rep2[:, DO:], op=AX.add)
                nc.vector.tensor_tensor(o[:], o[:], psR[:, 0:DO], op=AX.add)
                nc.vector.tensor_tensor(o[:], o[:], bres_sb[:], op=AX.add)
                nc.sync.dma_start(out[n0:n0 + 128, :], o[:])

    nc.compile()
    return nc


def kernel(**inputs):
    cfg = Cfg()
    in_maps, sch = host_prep(
        cfg,
        np.asarray(inputs["feat"], np.float32), np.asarray(inputs["edges"]),
        np.asarray(inputs["W_heads"], np.float32), np.asarray(inputs["a_heads"], np.float32),
        np.asarray(inputs["gamma_h"], np.float32), np.asarray(inputs["beta_h"], np.float32),
        np.asarray(inputs["W_out"], np.float32), np.asarray(inputs["a_out"], np.float32),
        np.asarray(inputs["gamma_o"], np.float32), np.asarray(inputs["beta_o"], np.float32),
        np.asarray(inputs["W_res"], np.float32), np.asarray(inputs["b_res"], np.float32))
    nc = build_module(cfg, sch)
    from concourse.bass_utils import run_bass_kernel_spmd
    res = run_bass_kernel_spmd(nc, in_maps, core_ids=list(range(cfg.CORES)))
    outs = [res.results[ci]["out"][:cfg.NLOC] for ci in range(cfg.CORES)]
    return np.concatenate(outs, axis=0).astype(np.float32)


if __name__ == "__main__":
    d = np.load("/root/problem/ref_cache.npz")
    got = kernel(**{k: d[k] for k in d.files if k != "expected"})
    exp = d["expected"]
    err = np.abs(got - exp).max() / np.abs(exp).max()
    print("scale-relative err:", err)
